# revision 22
# baseline (speedup 1.0000x reference)
"""Trainium2 Bass kernel for nn_Block_11166914969721 (dense transformer block).

Sharding: 8 cores = (batch b in {0,1}) x (query chunk qc in {0..3}, 1024
queries each). Each core recomputes the full KV side for its batch (LN1,
spatial-reduction convs, kv projections, depthwise convs) and computes
attention + proj + MLP for its own query chunk. The host does layout prep
(transposes, weight folding, diagonalized depthwise weights) and reassembles
the output.

Device layout convention: activations live in "T-layout" [channels
(partitions), tokens (free)] so matmuls contract channels on the partition
dim. Per-token LayerNorm stats are computed with ones-matmuls on the tensor
engine and broadcast to all partitions via a DRAM round-trip. The depthwise
3x3x3 convs run on the tensor engine as 27 PSUM-accumulated matmuls with
diagonal weights against shifted views of a zero-padded buffer. Attention
keeps S transposed ([keys, queries]); softmax denominators fall out of the
AV matmul via an appended ones-column on V.
"""

import numpy as np
import ml_dtypes

import concourse.bass as bass
import concourse.mybir as mybir
import concourse.tile as tile
from concourse.bass_utils import run_bass_kernel_spmd
from concourse.masks import make_identity
from concourse.vector_clock import ScopedClock

BF = ml_dtypes.bfloat16
AL = mybir.AluOpType
AF = mybir.ActivationFunctionType
F32 = mybir.dt.float32
BF16 = mybir.dt.bfloat16

# ---------------------------------------------------------------------------
# Workarounds: walrus in this container accepts at most ONE sem-wait per
# instruction. (a) Tile's kernel-tail drain aggregates one wait per live
# proc -> spread across SP nops. (b) Mid-kernel instructions may also get
# several waits -> post-pass splits them onto same-engine NoOps.
# ---------------------------------------------------------------------------


def _patched_drain_and_barrier(self, tick_clock, wait_clock):
    nc = self.nc
    collector = nc.sync.nop(nofuse=True)
    wait_clock.add_sem_waits(collector.ins, ScopedClock({None: tick_clock.global_clock}))
    si = collector.ins.sync_info
    waits = list(si.on_wait) if si is not None and si.on_wait else []
    if si is not None:
        si.on_wait = waits[:1]
    for i in range(1, len(waits)):
        nop = nc.sync.nop(nofuse=True)
        nop.ins.sync_info = mybir.SyncInfo(on_wait=waits[i:i + 1], on_update=[])
    nc.sync.drain()
    nc.all_engine_barrier()
    assert self.sems is not None
    popped = nc._tile_sem_poison_stack.pop()
    assert popped is self._sem_poison
    nc.clear_and_free_semaphores(list(self.sems.allocated().values()))
    nc.all_engine_barrier()


tile.TileContext._drain_and_barrier = _patched_drain_and_barrier


def _split_multi_waits(nc):
    cnt = 0
    for fn in nc.m.functions:
        for bb in fn.blocks:
            out = []
            for inst in bb.instructions:
                si = inst.sync_info
                if si is not None and si.on_wait and len(si.on_wait) > 1:
                    waits = list(si.on_wait)
                    for w in waits[:-1]:
                        cnt += 1
                        out.append(mybir.InstNoOp(
                            name=f"nwsplit{cnt}",
                            engine=inst.engine,
                            sync_info=mybir.SyncInfo(on_wait=[w], on_update=[]),
                            bass_nofuse=True))
                    si.on_wait = waits[-1:]
                out.append(inst)
            bb.instructions[:] = out
    return cnt


# ---------------------------------------------------------------------------
B, N, C = 2, 4096, 384
HD = 48
C2 = 192
N1 = 512
HID = 4 * C
NQ = 1024          # queries per core
CT = 3             # channel tiles of 128
EPS = 1e-5


def _bound_runs(lo, hi, bounds):
    bs = sorted(set([lo, hi] + [x for x in bounds if lo < x < hi]))
    return list(zip(bs, bs[1:]))


def _q_runs(mt):
    """q-proj psum co-tile mt -> (psum_row, len, branch, tile, dst_row)."""
    out = []
    for a, b in _bound_runs(mt * 128, mt * 128 + 128, [k * 48 for k in range(1, 8)]):
        h = a // 48
        out.append((a - mt * 128, b - a, h // 4, (h % 4) // 2,
                    64 * (h % 2) + (a - 48 * h)))
    return out


def _kv_runs(mt):
    """kv-proj psum co-tile mt -> ('k', row, len, tile, dst_row) / ('v', ...)."""
    out = []
    for a, b in _bound_runs(mt * 128, mt * 128 + 128, [48, 96, 144, 192, 320]):
        if a < 192:
            h = a // 48
            out.append(("k", a - mt * 128, b - a, h // 2, 64 * (h % 2) + (a - 48 * h)))
        else:
            vc = a - 192
            out.append(("v", a - mt * 128, b - a, int(vc >= 128), vc % 128))
    return out


def _v49_runs(c0, ln):
    """channel range of v -> 49-augmented column offsets: (src_off, len, dst_col)."""
    out = []
    for a, b in _bound_runs(c0, c0 + ln, [k * 48 for k in range(1, 4)]):
        out.append((a - c0, b - a, (a // 48) * 49 + a % 48))
    return out


def _cat_runs(c0, ln):
    """channel range -> (ct, dst_row, src_off, len) split at 128-boundaries."""
    out = []
    c = c0
    while c < c0 + ln:
        ct = c // 128
        e = min(c0 + ln, (ct + 1) * 128)
        out.append((ct, c - 128 * ct, c - c0, e - c))
        c = e
    return out


def _bcast(ap, p):
    """Replicate a 1-D AP across p partitions (partition-stride 0)."""
    return bass.AP(tensor=ap.tensor, offset=ap.offset, ap=[[0, p]] + list(ap.ap))


def build_program():
    nc = bass.Bass()
    d = {}

    def din(name, shape, dt):
        d[name] = nc.dram_tensor(name, shape, dt, kind="ExternalInput")

    din("xt", [C, N], BF16)
    din("xct", [C, NQ], F32)
    din("qw", [C, C], BF16)
    din("qb", [C], F32)
    din("s2w", [C, C], BF16)
    din("s2b", [C], F32)
    din("s1w", [8, C, C], BF16)
    din("s1b", [C], F32)
    din("n1w", [C], F32)
    din("n1b", [C], F32)
    din("n2w", [C], F32)
    din("n2b", [C], F32)
    din("kv1w", [C, C], BF16)
    din("kv2w", [C, C], BF16)
    din("dg1a", [128, 27, 128], BF16)
    din("dg1b", [64, 27, 64], BF16)
    din("dg2a", [128, 27, 128], BF16)
    din("dg2b", [64, 27, 64], BF16)
    din("lc1b", [C2], F32)
    din("lc2b", [C2], F32)
    din("pw", [C, C], BF16)
    din("pb", [C], F32)
    din("ln2w", [C], F32)
    din("ln2b", [C], F32)
    din("f1w", [C, HID], BF16)
    din("f1b", [HID], F32)
    din("f2w", [HID, C], BF16)
    din("f2b", [C], F32)

    out_d = nc.dram_tensor("out", [NQ, C], F32, kind="ExternalOutput")

    scr = {}
    for nm, ntok, adt in [("l1", N, BF16), ("n2", N, BF16), ("n1", N1, BF16),
                          ("lq", NQ, BF16), ("l2", NQ, F32)]:
        scr[nm] = (nc.dram_tensor(f"sx_{nm}", [ntok], F32),
                   nc.dram_tensor(f"sq_{nm}", [ntok], F32),
                   nc.dram_tensor(f"sa_{nm}", [ntok], adt),
                   nc.dram_tensor(f"sb_{nm}", [ntok], adt))
    dden = nc.dram_tensor("dden", [2, 2, 4, 512], F32)
    drec = nc.dram_tensor("drec", [2, 2, 4, 512], F32)

    with tile.TileContext(nc, pool_alloc_mode="queue") as tc:
        _body(tc, nc, d, out_d, scr, dden, drec)
    _split_multi_waits(nc)
    return nc


def _body(tc, nc, d, out_d, scr, dden, drec):
    from contextlib import ExitStack

    dma = nc.gpsimd.dma_start

    ctx = ExitStack()
    with ctx:
        glob = ctx.enter_context(tc.tile_pool(name="glob", bufs=1))
        wpool = ctx.enter_context(tc.tile_pool(name="wpool", bufs=1))
        rows = ctx.enter_context(tc.tile_pool(name="rows", bufs=1))
        stg = ctx.enter_context(tc.tile_pool(name="stg", bufs=2))
        pstat = ctx.enter_context(tc.tile_pool(name="pstat", bufs=1, space="PSUM"))

        ones_b = glob.tile([128, 1], BF16, tag="ones_b")
        nc.vector.memset(ones_b, 1.0)
        ones_f = glob.tile([128, 1], F32, tag="ones_f")
        nc.vector.memset(ones_f, 1.0)
        eps_t = glob.tile([128, 1], F32, tag="eps")
        nc.vector.memset(eps_t, EPS)
        ident = glob.tile([128, 128], BF16, tag="ident")
        make_identity(nc, ident)
        identf = glob.tile([128, 128], F32, tag="identf")
        make_identity(nc, identf)

        def vec_sb(name, nt=CT):
            t = wpool.tile([128, nt], F32, tag=f"v_{name}")
            dma(out=t, in_=d[name].rearrange("(t p) -> p t", p=128))
            return t

        def mat_sb(name, ktiles, cols, pool, tag=None):
            t = pool.tile([128, ktiles, cols], BF16, tag=tag or f"m_{name}", name=name)
            dma(out=t, in_=d[name].rearrange("(t p) co -> p t co", p=128))
            return t

        qb_sb = vec_sb("qb")
        s2b_sb = vec_sb("s2b")
        s1b_sb = vec_sb("s1b")
        n1w_sb = vec_sb("n1w")
        n1b_sb = vec_sb("n1b")
        n2w_sb = vec_sb("n2w")
        n2b_sb = vec_sb("n2b")
        pb_sb = vec_sb("pb")
        ln2w_sb = vec_sb("ln2w")
        ln2b_sb = vec_sb("ln2b")
        f2b_sb = vec_sb("f2b")
        f1b_sb = vec_sb("f1b", 12)

        lcb = {}
        for br, bn in ((1, "lc1b"), (2, "lc2b")):
            a = wpool.tile([128, 1], F32, tag=f"{bn}a")
            dma(out=a, in_=d[bn][0:128].rearrange("(p o) -> p o", o=1))
            b = wpool.tile([64, 1], F32, tag=f"{bn}b")
            dma(out=b, in_=d[bn][128:192].rearrange("(p o) -> p o", o=1))
            lcb[br] = (a, b)

        # ------------------------------------------------------------------
        def t_ln_rows(src, ntok, key, pool, src_f32=False, out_dt=BF16):
            """LN stats over channels (partitions) in T-layout -> broadcast
            rows a = rsqrt(var+eps), b = -mean*a as [128, ntok] tiles."""
            sxd, sqd, sad, sbd = scr[key]
            K = ntok // 128
            ones = ones_f if src_f32 else ones_b
            sq_dt = F32 if src_f32 else BF16
            for ch in range(ntok // 512):
                st = pstat.tile([64, 512], F32, tag="st")
                sqt = rows.tile([128, 512], sq_dt, tag=f"sqc{int(src_f32)}")
                for ct in range(CT):
                    nc.tensor.matmul(st[0:1, :], lhsT=ones,
                                     rhs=src(ct)[:, ch * 512:(ch + 1) * 512],
                                     start=(ct == 0), stop=(ct == CT - 1))
                for ct in range(CT):
                    nc.vector.tensor_mul(sqt, src(ct)[:, ch * 512:(ch + 1) * 512],
                                         src(ct)[:, ch * 512:(ch + 1) * 512])
                    nc.tensor.matmul(st[32:33, :], lhsT=ones, rhs=sqt,
                                     start=(ct == 0), stop=(ct == CT - 1))
                sts = rows.tile([64, 512], F32, tag="sts")
                nc.vector.tensor_copy(out=sts[0:33, :], in_=st[0:33, :])
                dma(out=sxd[ch * 512:(ch + 1) * 512], in_=sts[0:1, :])
                dma(out=sqd[ch * 512:(ch + 1) * 512], in_=sts[32:33, :])
            rsx = rows.tile([128, K], F32, tag="rsx")
            rsq = rows.tile([128, K], F32, tag="rsq")
            dma(out=rsx, in_=sxd.rearrange("(p k) -> p k", p=128))
            dma(out=rsq, in_=sqd.rearrange("(p k) -> p k", p=128))
            rm = rows.tile([128, K], F32, tag="rm")
            rv = rows.tile([128, K], F32, tag="rv")
            nc.vector.tensor_scalar_mul(out=rm, in0=rsx, scalar1=1.0 / C)
            nc.vector.tensor_scalar_mul(out=rsq, in0=rsq, scalar1=1.0 / C)
            nc.vector.tensor_mul(rv, rm, rm)
            nc.vector.tensor_sub(rv, rsq, rv)
            nc.scalar.activation(out=rv, in_=rv, func=AF.Sqrt, bias=eps_t)
            rg = rows.tile([128, K], F32, tag="rg")
            nc.vector.reciprocal(out=rg, in_=rv)
            ra = rows.tile([128, K], out_dt, tag="ra")
            nc.vector.tensor_copy(out=ra, in_=rg)
            rb = rows.tile([128, K], out_dt, tag="rb")
            nc.vector.scalar_tensor_tensor(out=rb, in0=rm, scalar=-1.0, in1=rg,
                                           op0=AL.mult, op1=AL.mult)
            dma(out=sad.rearrange("(p k) -> p k", p=128), in_=ra)
            dma(out=sbd.rearrange("(p k) -> p k", p=128), in_=rb)
            a_bc = pool.tile([128, ntok], out_dt, tag="abc", name=f"abc_{key}")
            b_bc = pool.tile([128, ntok], out_dt, tag="bbc", name=f"bbc_{key}")
            dma(out=a_bc, in_=_bcast(sad[:], 128))
            dma(out=b_bc, in_=_bcast(sbd[:], 128))
            return a_bc, b_bc

        # persistent activations
        q1p = glob.tile([128, 2, NQ], BF16, tag="q1p")
        q2p = glob.tile([128, 2, NQ], BF16, tag="q2p")
        k1p = glob.tile([128, 2, N1], BF16, tag="k1p")
        k2p = glob.tile([128, 2, N], BF16, tag="k2p")
        for t in (q1p, q2p, k1p, k2p):
            nc.vector.memset(t, 0.0)
        v1n = glob.tile([128, N1 // 128, 196], BF16, tag="v1n")
        v2n = glob.tile([128, N // 128, 196], BF16, tag="v2n")
        for vn in (v1n, v2n):
            for h in range(4):
                nc.vector.memset(vn[:, :, h * 49 + 48:h * 49 + 49], 1.0)
        ocat = glob.tile([128, CT, NQ], BF16, tag="ocat")

        # ============ LN1, projections, convs ============
        # One pool; sequential-lifetime tensors share tags (slots).
        with tc.tile_pool(name="big", bufs=1) as big, \
             tc.tile_pool(name="pmm", bufs=2, space="PSUM") as pmm, \
             tc.tile_pool(name="s1wp", bufs=2) as s1wp:
            xt_sb = big.tile([128, CT, N], BF16, tag="t_big1", name="xt_sb")
            dma(out=xt_sb, in_=d["xt"].rearrange("(t p) n -> p t n", p=128))
            a1, b1 = t_ln_rows(lambda ct: xt_sb[:, ct, :], N, "l1", big)
            xa = big.tile([128, CT, N], BF16, tag="t_big2", name="xa")
            for ct in range(CT):
                nc.vector.tensor_mul(xa[:, ct, :], xt_sb[:, ct, :], a1)
                nc.vector.tensor_add(xa[:, ct, :], xa[:, ct, :], b1)

            xct_sb = big.tile([128, CT, NQ], F32, tag="t_xct", name="xct_sb")
            dma(out=xct_sb, in_=d["xct"].rearrange("(t p) n -> p t n", p=128))
            aq, bq = t_ln_rows(lambda ct: xct_sb[:, ct, :], NQ, "lq", big,
                               src_f32=True)
            xaq = big.tile([128, CT, NQ], BF16, tag="t_xaq", name="xaq")
            for ct in range(CT):
                nc.vector.tensor_mul(xaq[:, ct, :], xct_sb[:, ct, :], aq)
                nc.vector.tensor_add(xaq[:, ct, :], xaq[:, ct, :], bq)

            # --- q projection (own chunk) ---
            qw_sb = mat_sb("qw", CT, C, big, "t_w1")
            for mt in range(CT):
                for ch in range(NQ // 512):
                    ps = pmm.tile([128, 512], F32, tag="mm")
                    for ct in range(CT):
                        nc.tensor.matmul(
                            ps, lhsT=qw_sb[:, ct, mt * 128:(mt + 1) * 128],
                            rhs=xaq[:, ct, ch * 512:(ch + 1) * 512],
                            start=(ct == 0), stop=(ct == CT - 1))
                    qs = stg.tile([128, 512], BF16, tag="qs")
                    nc.vector.tensor_scalar(out=qs, in0=ps,
                                            scalar1=qb_sb[:, mt:mt + 1],
                                            scalar2=None, op0=AL.add)
                    for (row, ln, br, tt, r0) in _q_runs(mt):
                        dst = q1p if br == 0 else q2p
                        dma(out=dst[r0:r0 + ln, tt, ch * 512:(ch + 1) * 512],
                            in_=qs[row:row + ln, :])

            # --- sr2 ---
            s2w_sb = mat_sb("s2w", CT, C, big, "t_w1")
            y2 = big.tile([128, CT, N], BF16, tag="t_big1", name="y2")
            for mt in range(CT):
                for ch in range(N // 512):
                    ps = pmm.tile([128, 512], F32, tag="mm")
                    for ct in range(CT):
                        nc.tensor.matmul(
                            ps, lhsT=s2w_sb[:, ct, mt * 128:(mt + 1) * 128],
                            rhs=xa[:, ct, ch * 512:(ch + 1) * 512],
                            start=(ct == 0), stop=(ct == CT - 1))
                    nc.vector.tensor_scalar(
                        out=y2[:, mt, ch * 512:(ch + 1) * 512],
                        in0=ps, scalar1=s2b_sb[:, mt:mt + 1],
                        scalar2=None, op0=AL.add)
            # --- sr1 ---
            y1 = big.tile([128, CT, N1], BF16, tag="t_y1", name="y1")
            with tc.tile_pool(name="ps1p", bufs=1, space="PSUM") as ps1p:
                ps1 = [ps1p.tile([128, 512], F32, tag=f"s1_{m}",
                                 name=f"ps1_{m}") for m in range(CT)]
                k = 0
                for oi in range(8):
                    a_, b_, c_ = oi // 4, (oi // 2) % 2, oi % 2
                    for ct in range(CT):
                        wt = s1wp.tile([128, C], BF16, tag="s1w")
                        dma(out=wt, in_=d["s1w"][oi, ct * 128:(ct + 1) * 128, :])
                        rhs = xa[:, ct, :].rearrange(
                            "p (h a w b d c) -> p a b c h w d",
                            h=8, a=2, w=8, b=2, d=8, c=2)[:, a_, b_, c_]
                        for mt in range(CT):
                            nc.tensor.matmul(
                                ps1[mt], lhsT=wt[:, mt * 128:(mt + 1) * 128],
                                rhs=rhs, start=(k == 0), stop=(k == 23))
                        k += 1
                for mt in range(CT):
                    nc.vector.tensor_scalar(out=y1[:, mt, :], in0=ps1[mt],
                                            scalar1=s1b_sb[:, mt:mt + 1],
                                            scalar2=None, op0=AL.add)

            # --- n2/n1 LN + gelu ---
            x2 = big.tile([128, CT, N], BF16, tag="t_big2", name="x2")
            a2, b2 = t_ln_rows(lambda ct: y2[:, ct, :], N, "n2", big)
            tmpn = big.tile([128, N], BF16, tag="t_vp", name="tmpn")
            for ct in range(CT):
                nc.vector.tensor_mul(tmpn, y2[:, ct, :], a2)
                nc.vector.tensor_add(tmpn, tmpn, b2)
                nc.scalar.activation(out=x2[:, ct, :], in_=tmpn, func=AF.Gelu,
                                     bias=n2b_sb[:, ct:ct + 1],
                                     scale=n2w_sb[:, ct:ct + 1])
            x1 = big.tile([128, CT, N1], BF16, tag="t_x1", name="x1")
            a1b, b1b = t_ln_rows(lambda ct: y1[:, ct, :], N1, "n1", big)
            tm1 = big.tile([128, N1], BF16, tag="t_tm1", name="tm1")
            for ct in range(CT):
                nc.vector.tensor_mul(tm1, y1[:, ct, :], a1b)
                nc.vector.tensor_add(tm1, tm1, b1b)
                nc.scalar.activation(out=x1[:, ct, :], in_=tm1, func=AF.Gelu,
                                     bias=n1b_sb[:, ct:ct + 1],
                                     scale=n1w_sb[:, ct:ct + 1])

            # --- kv projections ---
            kv1w_sb = mat_sb("kv1w", CT, C, big, "t_w1")
            kv2w_sb = mat_sb("kv2w", CT, C, big, "t_w2")
            v2t0 = big.tile([128, N], BF16, tag="t_big1", name="v2t0")
            v2t1 = big.tile([64, N], BF16, tag="t_xaq", name="v2t1")
            v1t0 = big.tile([128, N1], BF16, tag="t_y1", name="v1t0")
            v1t1 = big.tile([64, N1], BF16, tag="t_v1t1", name="v1t1")
            for (src, wsb, kp, vt0, vt1, ntok) in (
                    (x2, kv2w_sb, k2p, v2t0, v2t1, N),
                    (x1, kv1w_sb, k1p, v1t0, v1t1, N1)):
                for mt in range(CT):
                    for ch in range(ntok // 512):
                        ps = pmm.tile([128, 512], F32, tag="mm")
                        for ct in range(CT):
                            nc.tensor.matmul(
                                ps, lhsT=wsb[:, ct, mt * 128:(mt + 1) * 128],
                                rhs=src[:, ct, ch * 512:(ch + 1) * 512],
                                start=(ct == 0), stop=(ct == CT - 1))
                        ks = stg.tile([128, 512], BF16, tag="ks")
                        nc.vector.tensor_copy(out=ks, in_=ps)
                        for run in _kv_runs(mt):
                            if run[0] == "k":
                                _, row, ln, tt, r0 = run
                                dma(out=kp[r0:r0 + ln, tt, ch * 512:(ch + 1) * 512],
                                    in_=ks[row:row + ln, :])
                            else:
                                _, row, ln, vt, r0 = run
                                dst = vt0 if vt == 0 else vt1
                                dma(out=dst[r0:r0 + ln, ch * 512:(ch + 1) * 512],
                                    in_=ks[row:row + ln, :])

            # --- depthwise conv on PE + transpose into 49-augmented layout ---
            with tc.tile_pool(name="ptr", bufs=2, space="PSUM") as ptr:
                dgs = {}
                for nm, np2, tg in (("dg1a", 128, "t_tm1"), ("dg1b", 64, "t_x1"),
                                    ("dg2a", 128, "t_xct"), ("dg2b", 64, "t_dg2b")):
                    t = big.tile([np2, 27, np2], BF16, tag=tg, name=nm)
                    dma(out=t, in_=d[nm][:, :, :])
                    dgs[nm] = t
                offs = [(dz, dy, dx) for dz in range(3) for dy in range(3)
                        for dx in range(3)]
                for (br, vt0, vt1, vn, S, P) in ((2, v2t0, v2t1, v2n, 16, 18),
                                                 (1, v1t0, v1t1, v1n, 8, 10)):
                    ntok = S * S * S
                    for (half, vt, np_) in (("a", vt0, 128), ("b", vt1, 64)):
                        dgt = dgs[f"dg{br}{half}"]
                        bia = lcb[br][0 if half == "a" else 1]
                        vp = big.tile([np_, P * P * P], BF16,
                                      tag="t_vp",
                                      name=f"vp{br}{half}")
                        nc.vector.memset(vp, 0.0)
                        vpv = vp.rearrange("p (h w d) -> p h w d", h=P, w=P, d=P)
                        nc.vector.tensor_copy(
                            out=vpv[:, 1:S + 1, 1:S + 1, 1:S + 1],
                            in_=vt.rearrange("p (h w d) -> p h w d", h=S, w=S, d=S))
                        acc = big.tile([np_, ntok], BF16,
                                       tag="t_acc",
                                       name=f"ac{br}{half}")
                        hrows = 512 // (S * S)
                        for ch in range(ntok // 512):
                            pd_ = pmm.tile([128, 512], F32, tag="mm")
                            for j, (dz, dy, dx) in enumerate(offs):
                                rhs = bass.AP(
                                    tensor=vp.tensor,
                                    offset=vp.offset + ch * hrows * P * P
                                    + dz * P * P + dy * P + dx,
                                    ap=[list(vp.ap[0]), [P * P, hrows],
                                        [P, S], [1, S]])
                                nc.tensor.matmul(pd_[0:np_, :], lhsT=dgt[:, j, :],
                                                 rhs=rhs, start=(j == 0),
                                                 stop=(j == 26))
                            nc.vector.scalar_tensor_tensor(
                                out=acc[:, ch * 512:(ch + 1) * 512],
                                in0=pd_[0:np_, :], scalar=bia,
                                in1=vt[:, ch * 512:(ch + 1) * 512],
                                op0=AL.add, op1=AL.add)
                        for mt in range(ntok // 128):
                            tp = ptr.tile([128, 128], BF16, tag="tp")
                            nc.tensor.transpose(tp[:, 0:np_],
                                                acc[:, mt * 128:(mt + 1) * 128],
                                                ident[0:np_, 0:np_])
                            vst = stg.tile([128, 128], BF16, tag="vst")
                            nc.vector.tensor_copy(out=vst[:, 0:np_], in_=tp[:, 0:np_])
                            c0 = 0 if half == "a" else 128
                            for (soff, ln, dcol) in _v49_runs(c0, np_):
                                dma(out=vn[:, mt, dcol:dcol + ln],
                                    in_=vst[:, soff:soff + ln])

        # ================= attention =================
        with tc.tile_pool(name="pS", bufs=1, space="PSUM") as pS, \
             tc.tile_pool(name="pO", bufs=1, space="PSUM") as pO, \
             tc.tile_pool(name="pex", bufs=2) as pex, \
             tc.tile_pool(name="prec", bufs=2) as prec:
            for nb in range(NQ // 512):
                for (br, kp, qp, vn, nmt) in ((2, k2p, q2p, v2n, N // 128),
                                              (1, k1p, q1p, v1n, N1 // 128)):
                    oa = pO.tile([128, 512], F32, tag="oa")
                    ob = pO.tile([128, 512], F32, tag="ob")
                    for mt in range(nmt):
                        S = pS.tile([128, 2048], F32, tag="S")
                        for h in range(4):
                            tt, r = h // 2, h % 2
                            nc.tensor.matmul(
                                S[:, h * 512:(h + 1) * 512],
                                lhsT=kp[64 * r:64 * r + 64, tt, mt * 128:(mt + 1) * 128],
                                rhs=qp[64 * r:64 * r + 64, tt, nb * 512:(nb + 1) * 512],
                                start=True, stop=True, tile_position=(64 * r, 0))
                        ex = pex.tile([128, 2048], BF16, tag="ex")
                        nc.scalar.activation(out=ex, in_=S, func=AF.Exp)
                        for h in range(4):
                            ot = oa if h < 2 else ob
                            cp = 64 * (h % 2)
                            nc.tensor.matmul(ot[cp:cp + 49, :],
                                             lhsT=vn[:, mt, h * 49:(h + 1) * 49],
                                             rhs=ex[:, h * 512:(h + 1) * 512],
                                             start=(mt == 0), stop=(mt == nmt - 1),
                                             tile_position=(0, cp))
                    # denominators -> recip -> broadcast (DRAM round-trips)
                    dta = stg.tile([128, 512], F32, tag="dta")
                    nc.vector.tensor_copy(out=dta[0:32, :], in_=oa[32:64, :])
                    nc.vector.tensor_copy(out=dta[32:64, :], in_=oa[96:128, :])
                    nc.vector.tensor_copy(out=dta[64:96, :], in_=ob[32:64, :])
                    nc.vector.tensor_copy(out=dta[96:128, :], in_=ob[96:128, :])
                    for h, r0 in enumerate((16, 48, 80, 112)):
                        dma(out=dden[br - 1, nb, h], in_=dta[r0:r0 + 1, :])
                    r16 = prec.tile([128, 16], F32, tag="r16")
                    dma(out=r16, in_=dden[br - 1, nb].rearrange("h (p k) -> (h p) k", k=16))
                    rr = prec.tile([128, 16], F32, tag="rr")
                    nc.vector.reciprocal(out=rr, in_=r16)
                    dma(out=drec[br - 1, nb].rearrange("h (p k) -> (h p) k", k=16), in_=rr)
                    reca = prec.tile([128, 512], F32, tag="reca")
                    recb = prec.tile([128, 512], F32, tag="recb")
                    for h, rt in ((0, reca), (1, reca), (2, recb), (3, recb)):
                        dma(out=rt[64 * (h % 2):64 * (h % 2) + 48, :],
                            in_=_bcast(drec[br - 1, nb, h], 48))
                    # normalize into staging, then split-DMA into ocat
                    cbase = 0 if br == 1 else C2
                    for pi, (srcp, rt) in enumerate(((oa, reca), (ob, recb))):
                        ost = stg.tile([128, 512], BF16, tag="ost")
                        for r0 in (0, 64):
                            nc.vector.scalar_tensor_tensor(
                                out=ost[r0:r0 + 48, :], in0=srcp[r0:r0 + 48, :],
                                scalar=1.0, in1=rt[r0:r0 + 48, :],
                                op0=AL.mult, op1=AL.mult)
                        for hh in range(2):
                            h = pi * 2 + hh
                            for (ct, drow, off, ln) in _cat_runs(cbase + h * 48, 48):
                                dma(out=ocat[drow:drow + ln, ct,
                                             nb * 512:(nb + 1) * 512],
                                    in_=ost[64 * hh + off:64 * hh + off + ln, :])

        # ================= proj + LN2 + MLP + output =================
        with tc.tile_pool(name="pG", bufs=1) as pG, \
             tc.tile_pool(name="pmm2", bufs=3, space="PSUM") as pmm2, \
             tc.tile_pool(name="ptr2", bufs=2, space="PSUM") as ptr2:
            pw_sb = mat_sb("pw", CT, C, pG)
            f1w_sb = mat_sb("f1w", CT, HID, pG)
            f2w_sb = mat_sb("f2w", 12, C, pG)
            zt = pG.tile([128, CT, NQ], F32, tag="zt")
            xct_sb = pG.tile([128, CT, NQ], F32, tag="xctG")
            dma(out=xct_sb, in_=d["xct"].rearrange("(t p) n -> p t n", p=128))
            for mt in range(CT):
                for ch in range(NQ // 512):
                    ps = pmm2.tile([128, 512], F32, tag="mm")
                    for ct in range(CT):
                        nc.tensor.matmul(ps, lhsT=pw_sb[:, ct, mt * 128:(mt + 1) * 128],
                                         rhs=ocat[:, ct, ch * 512:(ch + 1) * 512],
                                         start=(ct == 0), stop=(ct == CT - 1))
                    nc.vector.scalar_tensor_tensor(
                        out=zt[:, mt, ch * 512:(ch + 1) * 512], in0=ps,
                        scalar=pb_sb[:, mt:mt + 1],
                        in1=xct_sb[:, mt, ch * 512:(ch + 1) * 512],
                        op0=AL.add, op1=AL.add)
            a3, b3 = t_ln_rows(lambda ct: zt[:, ct, :], NQ, "l2", pG,
                               src_f32=True, out_dt=F32)
            xm = pG.tile([128, CT, NQ], BF16, tag="xm")
            tmp3 = pG.tile([128, NQ], F32, tag="tmp3")
            for ct in range(CT):
                nc.vector.tensor_mul(tmp3, zt[:, ct, :], a3)
                nc.vector.tensor_add(tmp3, tmp3, b3)
                nc.scalar.activation(out=xm[:, ct, :], in_=tmp3, func=AF.Identity,
                                     bias=ln2b_sb[:, ct:ct + 1],
                                     scale=ln2w_sb[:, ct:ct + 1])
            h1 = pG.tile([128, 12, NQ], BF16, tag="h1")
            for mt in range(12):
                for ch in range(NQ // 512):
                    ps = pmm2.tile([128, 512], F32, tag="mm")
                    for ct in range(CT):
                        nc.tensor.matmul(ps, lhsT=f1w_sb[:, ct, mt * 128:(mt + 1) * 128],
                                         rhs=xm[:, ct, ch * 512:(ch + 1) * 512],
                                         start=(ct == 0), stop=(ct == CT - 1))
                    nc.scalar.activation(out=h1[:, mt, ch * 512:(ch + 1) * 512],
                                         in_=ps, func=AF.Gelu,
                                         bias=f1b_sb[:, mt:mt + 1])
            for mt in range(CT):
                for ch in range(NQ // 512):
                    ps = pmm2.tile([128, 512], F32, tag="mm")
                    for kt in range(12):
                        nc.tensor.matmul(ps, lhsT=f2w_sb[:, kt, mt * 128:(mt + 1) * 128],
                                         rhs=h1[:, kt, ch * 512:(ch + 1) * 512],
                                         start=(kt == 0), stop=(kt == 11))
                    nc.vector.scalar_tensor_tensor(
                        out=zt[:, mt, ch * 512:(ch + 1) * 512], in0=ps,
                        scalar=f2b_sb[:, mt:mt + 1],
                        in1=zt[:, mt, ch * 512:(ch + 1) * 512],
                        op0=AL.add, op1=AL.add)
            for nt in range(NQ // 128):
                for ct in range(CT):
                    tp = ptr2.tile([128, 128], F32, tag="tp2")
                    nc.tensor.transpose(tp, zt[:, ct, nt * 128:(nt + 1) * 128], identf)
                    ots = stg.tile([128, 128], F32, tag="ots")
                    nc.vector.tensor_copy(out=ots, in_=tp)
                    dma(out=out_d[nt * 128:(nt + 1) * 128, ct * 128:(ct + 1) * 128],
                        in_=ots)


_PROG = None


def _get_program():
    global _PROG
    if _PROG is None:
        _PROG = build_program()
    return _PROG


def _diag(w):
    """[n, 27] weights -> [n, 27, n] per-offset diagonal matrices (bf16)."""
    n = w.shape[0]
    out = np.zeros((n, 27, n), BF)
    idx = np.arange(n)
    for j in range(27):
        out[idx, j, idx] = w[:, j].astype(BF)
    return out


def kernel(x, ln1_w, ln1_b, q_w, sr1_w, sr1_b, n1_w, n1_b, sr2_w, sr2_b,
           n2_w, n2_b, kv1_w, kv2_w, lc1_w, lc1_b, lc2_w, lc2_b,
           proj_w, proj_b, ln2_w, ln2_b, fc1_w, fc1_b, fc2_w, fc2_b,
           H, W, D):
    f = lambda a: np.asarray(a, np.float32)
    x = f(x)
    ln1_w, ln1_b = f(ln1_w), f(ln1_b)
    qs = HD ** -0.5
    lc1 = f(lc1_w).reshape(C2, 27)
    lc2 = f(lc2_w).reshape(C2, 27)

    wm = {
        "qw": np.ascontiguousarray((f(q_w) * ln1_w[None, :]).T * qs).astype(BF),
        "qb": (f(q_w) @ ln1_b * qs).astype(np.float32),
        "s2w": np.ascontiguousarray((f(sr2_w)[:, :, 0, 0, 0] * ln1_w[None, :]).T).astype(BF),
        "s2b": (f(sr2_b) + f(sr2_w)[:, :, 0, 0, 0] @ ln1_b).astype(np.float32),
        "s1w": np.ascontiguousarray(
            (f(sr1_w) * ln1_w[None, :, None, None, None])
            .transpose(2, 3, 4, 1, 0).reshape(8, C, C)).astype(BF),
        "s1b": (f(sr1_b) + np.einsum("ocijk,c->o", f(sr1_w), ln1_b)).astype(np.float32),
        "n1w": f(n1_w), "n1b": f(n1_b), "n2w": f(n2_w), "n2b": f(n2_b),
        "kv1w": np.ascontiguousarray(f(kv1_w).T).astype(BF),
        "kv2w": np.ascontiguousarray(f(kv2_w).T).astype(BF),
        "dg1a": _diag(lc1[0:128]), "dg1b": _diag(lc1[128:192]),
        "dg2a": _diag(lc2[0:128]), "dg2b": _diag(lc2[128:192]),
        "lc1b": f(lc1_b), "lc2b": f(lc2_b),
        "pw": np.ascontiguousarray(f(proj_w).T).astype(BF),
        "pb": f(proj_b),
        "ln2w": f(ln2_w), "ln2b": f(ln2_b),
        "f1w": np.ascontiguousarray(f(fc1_w).T).astype(BF),
        "f1b": f(fc1_b),
        "f2w": np.ascontiguousarray(f(fc2_w).T).astype(BF),
        "f2b": f(fc2_b),
    }

    in_maps = []
    for core in range(8):
        b, qc = core // 4, core % 4
        xtb = x[b].T
        m = dict(wm)
        m["xt"] = np.ascontiguousarray(xtb).astype(BF)
        m["xct"] = np.ascontiguousarray(xtb[:, qc * NQ:(qc + 1) * NQ]).astype(np.float32)
        in_maps.append(m)

    nc = _get_program()
    res = run_bass_kernel_spmd(nc, in_maps, list(range(8)))

    out = np.empty((B, N, C), np.float32)
    for core in range(8):
        b, qc = core // 4, core % 4
        out[b, qc * NQ:(qc + 1) * NQ, :] = res.results[core]["out"]
    return out


# revision 24
# speedup vs baseline: 1788.4565x; 1788.4565x over previous
"""Trainium2 Bass kernel for nn_Block_11166914969721 (dense transformer block).

Sharding: 8 cores = (batch b in {0,1}) x (query chunk qc in {0..3}, 1024
queries each). Each core recomputes the full KV side for its batch (LN1,
spatial-reduction convs, kv projections, depthwise convs) and computes
attention + proj + MLP for its own query chunk. The host does layout prep
(transposes, weight folding, diagonalized depthwise weights) and reassembles
the output.

Device layout convention: activations live in "T-layout" [channels
(partitions), tokens (free)] so matmuls contract channels on the partition
dim. Per-token LayerNorm stats are computed with ones-matmuls on the tensor
engine and broadcast to all partitions via a DRAM round-trip. The depthwise
3x3x3 convs run on the tensor engine as 27 PSUM-accumulated matmuls with
diagonal weights against shifted views of a zero-padded buffer. Attention
keeps S transposed ([keys, queries]); softmax denominators fall out of the
AV matmul via an appended ones-column on V.
"""

import numpy as np
import ml_dtypes

import concourse.bass as bass
import concourse.mybir as mybir
import concourse.tile as tile
from concourse.bass_utils import run_bass_kernel_spmd
from concourse.masks import make_identity
from concourse.vector_clock import ScopedClock

BF = ml_dtypes.bfloat16
AL = mybir.AluOpType
AF = mybir.ActivationFunctionType
F32 = mybir.dt.float32
BF16 = mybir.dt.bfloat16

# ---------------------------------------------------------------------------
# Workarounds: walrus in this container accepts at most ONE sem-wait per
# instruction. (a) Tile's kernel-tail drain aggregates one wait per live
# proc -> spread across SP nops. (b) Mid-kernel instructions may also get
# several waits -> post-pass splits them onto same-engine NoOps.
# ---------------------------------------------------------------------------


def _patched_drain_and_barrier(self, tick_clock, wait_clock):
    nc = self.nc
    collector = nc.sync.nop(nofuse=True)
    wait_clock.add_sem_waits(collector.ins, ScopedClock({None: tick_clock.global_clock}))
    si = collector.ins.sync_info
    waits = list(si.on_wait) if si is not None and si.on_wait else []
    if si is not None:
        si.on_wait = waits[:1]
    for i in range(1, len(waits)):
        nop = nc.sync.nop(nofuse=True)
        nop.ins.sync_info = mybir.SyncInfo(on_wait=waits[i:i + 1], on_update=[])
    nc.sync.drain()
    nc.all_engine_barrier()
    assert self.sems is not None
    popped = nc._tile_sem_poison_stack.pop()
    assert popped is self._sem_poison
    nc.clear_and_free_semaphores(list(self.sems.allocated().values()))
    nc.all_engine_barrier()


tile.TileContext._drain_and_barrier = _patched_drain_and_barrier


def _split_multi_waits(nc):
    cnt = 0
    for fn in nc.m.functions:
        for bb in fn.blocks:
            out = []
            for inst in bb.instructions:
                si = inst.sync_info
                if si is not None and si.on_wait and len(si.on_wait) > 1:
                    waits = list(si.on_wait)
                    for w in waits[:-1]:
                        cnt += 1
                        out.append(mybir.InstNoOp(
                            name=f"nwsplit{cnt}",
                            engine=inst.engine,
                            sync_info=mybir.SyncInfo(on_wait=[w], on_update=[]),
                            bass_nofuse=True))
                    si.on_wait = waits[-1:]
                out.append(inst)
            bb.instructions[:] = out
    return cnt


# ---------------------------------------------------------------------------
B, N, C = 2, 4096, 384
HD = 48
C2 = 192
N1 = 512
HID = 4 * C
NQ = 1024          # queries per core
CT = 3             # channel tiles of 128
EPS = 1e-5


def _bound_runs(lo, hi, bounds):
    bs = sorted(set([lo, hi] + [x for x in bounds if lo < x < hi]))
    return list(zip(bs, bs[1:]))


def _q_runs(mt):
    """q-proj psum co-tile mt -> (psum_row, len, branch, tile, dst_row)."""
    out = []
    for a, b in _bound_runs(mt * 128, mt * 128 + 128, [k * 48 for k in range(1, 8)]):
        h = a // 48
        out.append((a - mt * 128, b - a, h // 4, (h % 4) // 2,
                    64 * (h % 2) + (a - 48 * h)))
    return out


def _kv_runs(mt):
    """kv-proj psum co-tile mt -> ('k', row, len, tile, dst_row) / ('v', ...)."""
    out = []
    for a, b in _bound_runs(mt * 128, mt * 128 + 128, [48, 96, 144, 192, 320]):
        if a < 192:
            h = a // 48
            out.append(("k", a - mt * 128, b - a, h // 2, 64 * (h % 2) + (a - 48 * h)))
        else:
            vc = a - 192
            out.append(("v", a - mt * 128, b - a, int(vc >= 128), vc % 128))
    return out


def _v49_runs(c0, ln):
    """channel range of v -> 49-augmented column offsets: (src_off, len, dst_col)."""
    out = []
    for a, b in _bound_runs(c0, c0 + ln, [k * 48 for k in range(1, 4)]):
        out.append((a - c0, b - a, (a // 48) * 49 + a % 48))
    return out


def _cat_runs(c0, ln):
    """channel range -> (ct, dst_row, src_off, len) split at 128-boundaries."""
    out = []
    c = c0
    while c < c0 + ln:
        ct = c // 128
        e = min(c0 + ln, (ct + 1) * 128)
        out.append((ct, c - 128 * ct, c - c0, e - c))
        c = e
    return out


def _bcast(ap, p):
    """Replicate a 1-D AP across p partitions (partition-stride 0)."""
    return bass.AP(tensor=ap.tensor, offset=ap.offset, ap=[[0, p]] + list(ap.ap))


def build_program():
    nc = bass.Bass()
    d = {}

    def din(name, shape, dt):
        d[name] = nc.dram_tensor(name, shape, dt, kind="ExternalInput")

    din("xt", [C, N], BF16)
    din("xct", [C, NQ], F32)
    din("qw", [C, C], BF16)
    din("qb", [C], F32)
    din("s2w", [C, C], BF16)
    din("s2b", [C], F32)
    din("s1w", [8, C, C], BF16)
    din("s1b", [C], F32)
    din("n1w", [C], F32)
    din("n1b", [C], F32)
    din("n2w", [C], F32)
    din("n2b", [C], F32)
    din("kv1w", [C, C], BF16)
    din("kv2w", [C, C], BF16)
    din("dg1a", [128, 27, 128], BF16)
    din("dg1b", [64, 27, 64], BF16)
    din("dg2a", [128, 27, 128], BF16)
    din("dg2b", [64, 27, 64], BF16)
    din("lc1b", [C2], F32)
    din("lc2b", [C2], F32)
    din("pw", [C, C], BF16)
    din("pb", [C], F32)
    din("ln2w", [C], F32)
    din("ln2b", [C], F32)
    din("f1w", [C, HID], BF16)
    din("f1b", [HID], F32)
    din("f2w", [HID, C], BF16)
    din("f2b", [C], F32)

    out_d = nc.dram_tensor("out", [NQ, C], F32, kind="ExternalOutput")

    scr = {}
    for nm, ntok, adt in [("l1", N, BF16), ("n2", N, BF16), ("n1", N1, BF16),
                          ("lq", NQ, BF16), ("l2", NQ, F32)]:
        scr[nm] = (nc.dram_tensor(f"sx_{nm}", [ntok], F32),
                   nc.dram_tensor(f"sq_{nm}", [ntok], F32),
                   nc.dram_tensor(f"sa_{nm}", [ntok], adt),
                   nc.dram_tensor(f"sb_{nm}", [ntok], adt))
    dden = nc.dram_tensor("dden", [2, 2, 4, 512], F32)
    drec = nc.dram_tensor("drec", [2, 2, 4, 512], F32)

    with tile.TileContext(nc, pool_alloc_mode="queue") as tc:
        _body(tc, nc, d, out_d, scr, dden, drec)
    _split_multi_waits(nc)
    return nc


def _body(tc, nc, d, out_d, scr, dden, drec):
    from contextlib import ExitStack

    dma = nc.gpsimd.dma_start

    ctx = ExitStack()
    with ctx:
        glob = ctx.enter_context(tc.tile_pool(name="glob", bufs=1))
        wpool = ctx.enter_context(tc.tile_pool(name="wpool", bufs=1))
        rows = ctx.enter_context(tc.tile_pool(name="rows", bufs=1))
        stg = ctx.enter_context(tc.tile_pool(name="stg", bufs=2))
        pstat = ctx.enter_context(tc.tile_pool(name="pstat", bufs=1, space="PSUM"))

        ones_b = glob.tile([128, 1], BF16, tag="ones_b")
        nc.vector.memset(ones_b, 1.0)
        ones_f = glob.tile([128, 1], F32, tag="ones_f")
        nc.vector.memset(ones_f, 1.0)
        eps_t = glob.tile([128, 1], F32, tag="eps")
        nc.vector.memset(eps_t, EPS)
        ident = glob.tile([128, 128], BF16, tag="ident")
        make_identity(nc, ident)
        identf = glob.tile([128, 128], F32, tag="identf")
        make_identity(nc, identf)

        def vec_sb(name, nt=CT):
            t = wpool.tile([128, nt], F32, tag=f"v_{name}")
            dma(out=t, in_=d[name].rearrange("(t p) -> p t", p=128))
            return t

        def mat_sb(name, ktiles, cols, pool, tag=None):
            t = pool.tile([128, ktiles, cols], BF16, tag=tag or f"m_{name}", name=name)
            dma(out=t, in_=d[name].rearrange("(t p) co -> p t co", p=128))
            return t

        qb_sb = vec_sb("qb")
        s2b_sb = vec_sb("s2b")
        s1b_sb = vec_sb("s1b")
        n1w_sb = vec_sb("n1w")
        n1b_sb = vec_sb("n1b")
        n2w_sb = vec_sb("n2w")
        n2b_sb = vec_sb("n2b")
        pb_sb = vec_sb("pb")
        ln2w_sb = vec_sb("ln2w")
        ln2b_sb = vec_sb("ln2b")
        f2b_sb = vec_sb("f2b")
        f1b_sb = vec_sb("f1b", 12)

        lcb = {}
        for br, bn in ((1, "lc1b"), (2, "lc2b")):
            a = wpool.tile([128, 1], F32, tag=f"{bn}a")
            dma(out=a, in_=d[bn][0:128].rearrange("(p o) -> p o", o=1))
            b = wpool.tile([64, 1], F32, tag=f"{bn}b")
            dma(out=b, in_=d[bn][128:192].rearrange("(p o) -> p o", o=1))
            lcb[br] = (a, b)

        # ------------------------------------------------------------------
        def t_ln_rows(src, ntok, key, pool, src_f32=False, out_dt=BF16):
            """LN stats over channels (partitions) in T-layout -> broadcast
            rows a = rsqrt(var+eps), b = -mean*a as [128, ntok] tiles."""
            sxd, sqd, sad, sbd = scr[key]
            K = ntok // 128
            ones = ones_f if src_f32 else ones_b
            sq_dt = F32 if src_f32 else BF16
            for ch in range(ntok // 512):
                st = pstat.tile([64, 512], F32, tag="st")
                sqt = rows.tile([128, 512], sq_dt, tag=f"sqc{int(src_f32)}")
                for ct in range(CT):
                    nc.tensor.matmul(st[0:1, :], lhsT=ones,
                                     rhs=src(ct)[:, ch * 512:(ch + 1) * 512],
                                     start=(ct == 0), stop=(ct == CT - 1))
                for ct in range(CT):
                    nc.vector.tensor_mul(sqt, src(ct)[:, ch * 512:(ch + 1) * 512],
                                         src(ct)[:, ch * 512:(ch + 1) * 512])
                    nc.tensor.matmul(st[32:33, :], lhsT=ones, rhs=sqt,
                                     start=(ct == 0), stop=(ct == CT - 1))
                sts = rows.tile([64, 512], F32, tag="sts")
                nc.vector.tensor_copy(out=sts[0:33, :], in_=st[0:33, :])
                dma(out=sxd[ch * 512:(ch + 1) * 512], in_=sts[0:1, :])
                dma(out=sqd[ch * 512:(ch + 1) * 512], in_=sts[32:33, :])
            rsx = rows.tile([128, K], F32, tag="rsx")
            rsq = rows.tile([128, K], F32, tag="rsq")
            dma(out=rsx, in_=sxd.rearrange("(p k) -> p k", p=128))
            dma(out=rsq, in_=sqd.rearrange("(p k) -> p k", p=128))
            rm = rows.tile([128, K], F32, tag="rm")
            rv = rows.tile([128, K], F32, tag="rv")
            nc.vector.tensor_scalar_mul(out=rm, in0=rsx, scalar1=1.0 / C)
            nc.vector.tensor_scalar_mul(out=rsq, in0=rsq, scalar1=1.0 / C)
            nc.vector.tensor_mul(rv, rm, rm)
            nc.vector.tensor_sub(rv, rsq, rv)
            nc.scalar.activation(out=rv, in_=rv, func=AF.Sqrt, bias=eps_t)
            rg = rows.tile([128, K], F32, tag="rg")
            nc.vector.reciprocal(out=rg, in_=rv)
            ra = rows.tile([128, K], out_dt, tag="ra")
            nc.vector.tensor_copy(out=ra, in_=rg)
            rb = rows.tile([128, K], out_dt, tag="rb")
            nc.vector.scalar_tensor_tensor(out=rb, in0=rm, scalar=-1.0, in1=rg,
                                           op0=AL.mult, op1=AL.mult)
            dma(out=sad.rearrange("(p k) -> p k", p=128), in_=ra)
            dma(out=sbd.rearrange("(p k) -> p k", p=128), in_=rb)
            a_bc = pool.tile([128, ntok], out_dt, tag="abc", name=f"abc_{key}")
            b_bc = pool.tile([128, ntok], out_dt, tag="bbc", name=f"bbc_{key}")
            dma(out=a_bc, in_=_bcast(sad[:], 128))
            dma(out=b_bc, in_=_bcast(sbd[:], 128))
            return a_bc, b_bc

        # persistent activations
        q1p = glob.tile([128, 2, NQ], BF16, tag="q1p")
        q2p = glob.tile([128, 2, NQ], BF16, tag="q2p")
        k1p = glob.tile([128, 2, N1], BF16, tag="k1p")
        k2p = glob.tile([128, 2, N], BF16, tag="k2p")
        for t in (q1p, q2p, k1p, k2p):
            nc.vector.memset(t, 0.0)
        v1n = glob.tile([128, N1 // 128, 196], BF16, tag="v1n")
        v2n = glob.tile([128, N // 128, 196], BF16, tag="v2n")
        for vn in (v1n, v2n):
            for h in range(4):
                nc.vector.memset(vn[:, :, h * 49 + 48:h * 49 + 49], 1.0)
        ocat = glob.tile([128, CT, NQ], BF16, tag="ocat")

        # ============ LN1, projections, convs ============
        # One pool; sequential-lifetime tensors share tags (slots).
        with tc.tile_pool(name="big", bufs=1) as big, \
             tc.tile_pool(name="pmm", bufs=2, space="PSUM") as pmm, \
             tc.tile_pool(name="s1wp", bufs=2) as s1wp:
            xt_sb = big.tile([128, CT, N], BF16, tag="t_big1", name="xt_sb")
            dma(out=xt_sb, in_=d["xt"].rearrange("(t p) n -> p t n", p=128))
            a1, b1 = t_ln_rows(lambda ct: xt_sb[:, ct, :], N, "l1", big)
            xa = big.tile([128, CT, N], BF16, tag="t_big2", name="xa")
            for ct in range(CT):
                nc.vector.tensor_mul(xa[:, ct, :], xt_sb[:, ct, :], a1)
                nc.vector.tensor_add(xa[:, ct, :], xa[:, ct, :], b1)

            xct_sb = big.tile([128, CT, NQ], F32, tag="t_xct", name="xct_sb")
            dma(out=xct_sb, in_=d["xct"].rearrange("(t p) n -> p t n", p=128))
            aq, bq = t_ln_rows(lambda ct: xct_sb[:, ct, :], NQ, "lq", big,
                               src_f32=True)
            xaq = big.tile([128, CT, NQ], BF16, tag="t_xaq", name="xaq")
            for ct in range(CT):
                nc.vector.tensor_mul(xaq[:, ct, :], xct_sb[:, ct, :], aq)
                nc.vector.tensor_add(xaq[:, ct, :], xaq[:, ct, :], bq)

            # --- q projection (own chunk) ---
            qw_sb = mat_sb("qw", CT, C, big, "t_w1")
            for mt in range(CT):
                for ch in range(NQ // 512):
                    ps = pmm.tile([128, 512], F32, tag="mm")
                    for ct in range(CT):
                        nc.tensor.matmul(
                            ps, lhsT=qw_sb[:, ct, mt * 128:(mt + 1) * 128],
                            rhs=xaq[:, ct, ch * 512:(ch + 1) * 512],
                            start=(ct == 0), stop=(ct == CT - 1))
                    qs = stg.tile([128, 512], BF16, tag="qs")
                    nc.vector.tensor_scalar(out=qs, in0=ps,
                                            scalar1=qb_sb[:, mt:mt + 1],
                                            scalar2=None, op0=AL.add)
                    for (row, ln, br, tt, r0) in _q_runs(mt):
                        dst = q1p if br == 0 else q2p
                        dma(out=dst[r0:r0 + ln, tt, ch * 512:(ch + 1) * 512],
                            in_=qs[row:row + ln, :])

            # --- sr2 ---
            s2w_sb = mat_sb("s2w", CT, C, big, "t_w1")
            y2 = big.tile([128, CT, N], BF16, tag="t_big1", name="y2")
            for mt in range(CT):
                for ch in range(N // 512):
                    ps = pmm.tile([128, 512], F32, tag="mm")
                    for ct in range(CT):
                        nc.tensor.matmul(
                            ps, lhsT=s2w_sb[:, ct, mt * 128:(mt + 1) * 128],
                            rhs=xa[:, ct, ch * 512:(ch + 1) * 512],
                            start=(ct == 0), stop=(ct == CT - 1))
                    nc.vector.tensor_scalar(
                        out=y2[:, mt, ch * 512:(ch + 1) * 512],
                        in0=ps, scalar1=s2b_sb[:, mt:mt + 1],
                        scalar2=None, op0=AL.add)
            # --- sr1 ---
            y1 = big.tile([128, CT, N1], BF16, tag="t_y1", name="y1")
            with tc.tile_pool(name="ps1p", bufs=1, space="PSUM") as ps1p:
                ps1 = [ps1p.tile([128, 512], F32, tag=f"s1_{m}",
                                 name=f"ps1_{m}") for m in range(CT)]
                k = 0
                for oi in range(8):
                    a_, b_, c_ = oi // 4, (oi // 2) % 2, oi % 2
                    for ct in range(CT):
                        wt = s1wp.tile([128, C], BF16, tag="s1w")
                        dma(out=wt, in_=d["s1w"][oi, ct * 128:(ct + 1) * 128, :])
                        rhs = xa[:, ct, :].rearrange(
                            "p (h a w b d c) -> p a b c h w d",
                            h=8, a=2, w=8, b=2, d=8, c=2)[:, a_, b_, c_]
                        for mt in range(CT):
                            nc.tensor.matmul(
                                ps1[mt], lhsT=wt[:, mt * 128:(mt + 1) * 128],
                                rhs=rhs, start=(k == 0), stop=(k == 23))
                        k += 1
                for mt in range(CT):
                    nc.vector.tensor_scalar(out=y1[:, mt, :], in0=ps1[mt],
                                            scalar1=s1b_sb[:, mt:mt + 1],
                                            scalar2=None, op0=AL.add)

            # --- n2/n1 LN + gelu ---
            x2 = big.tile([128, CT, N], BF16, tag="t_big2", name="x2")
            a2, b2 = t_ln_rows(lambda ct: y2[:, ct, :], N, "n2", big)
            tmpn = big.tile([128, N], BF16, tag="t_vp", name="tmpn")
            for ct in range(CT):
                nc.vector.tensor_mul(tmpn, y2[:, ct, :], a2)
                nc.vector.tensor_add(tmpn, tmpn, b2)
                nc.scalar.activation(out=x2[:, ct, :], in_=tmpn, func=AF.Gelu,
                                     bias=n2b_sb[:, ct:ct + 1],
                                     scale=n2w_sb[:, ct:ct + 1])
            x1 = big.tile([128, CT, N1], BF16, tag="t_x1", name="x1")
            a1b, b1b = t_ln_rows(lambda ct: y1[:, ct, :], N1, "n1", big)
            tm1 = big.tile([128, N1], BF16, tag="t_tm1", name="tm1")
            for ct in range(CT):
                nc.vector.tensor_mul(tm1, y1[:, ct, :], a1b)
                nc.vector.tensor_add(tm1, tm1, b1b)
                nc.scalar.activation(out=x1[:, ct, :], in_=tm1, func=AF.Gelu,
                                     bias=n1b_sb[:, ct:ct + 1],
                                     scale=n1w_sb[:, ct:ct + 1])

            # --- kv projections ---
            kv1w_sb = mat_sb("kv1w", CT, C, big, "t_w1")
            kv2w_sb = mat_sb("kv2w", CT, C, big, "t_w2")
            v2t0 = big.tile([128, N], BF16, tag="t_big1", name="v2t0")
            v2t1 = big.tile([64, N], BF16, tag="t_xaq", name="v2t1")
            v1t0 = big.tile([128, N1], BF16, tag="t_y1", name="v1t0")
            v1t1 = big.tile([64, N1], BF16, tag="t_v1t1", name="v1t1")
            for (src, wsb, kp, vt0, vt1, ntok) in (
                    (x2, kv2w_sb, k2p, v2t0, v2t1, N),
                    (x1, kv1w_sb, k1p, v1t0, v1t1, N1)):
                for mt in range(CT):
                    for ch in range(ntok // 512):
                        ps = pmm.tile([128, 512], F32, tag="mm")
                        for ct in range(CT):
                            nc.tensor.matmul(
                                ps, lhsT=wsb[:, ct, mt * 128:(mt + 1) * 128],
                                rhs=src[:, ct, ch * 512:(ch + 1) * 512],
                                start=(ct == 0), stop=(ct == CT - 1))
                        ks = stg.tile([128, 512], BF16, tag="ks")
                        nc.vector.tensor_copy(out=ks, in_=ps)
                        for run in _kv_runs(mt):
                            if run[0] == "k":
                                _, row, ln, tt, r0 = run
                                dma(out=kp[r0:r0 + ln, tt, ch * 512:(ch + 1) * 512],
                                    in_=ks[row:row + ln, :])
                            else:
                                _, row, ln, vt, r0 = run
                                dst = vt0 if vt == 0 else vt1
                                dma(out=dst[r0:r0 + ln, ch * 512:(ch + 1) * 512],
                                    in_=ks[row:row + ln, :])

            # --- depthwise conv on PE + transpose into 49-augmented layout ---
            with tc.tile_pool(name="ptr", bufs=2, space="PSUM") as ptr:
                dgs = {}
                for nm, np2, tg in (("dg1a", 128, "t_tm1"), ("dg1b", 64, "t_x1"),
                                    ("dg2a", 128, "t_xct"), ("dg2b", 64, "t_dg2b")):
                    t = big.tile([np2, 27, np2], BF16, tag=tg, name=nm)
                    dma(out=t, in_=d[nm][:, :, :])
                    dgs[nm] = t
                offs = [(dz, dy, dx) for dz in range(3) for dy in range(3)
                        for dx in range(3)]
                for (br, vt0, vt1, vn, S, P) in ((2, v2t0, v2t1, v2n, 16, 18),
                                                 (1, v1t0, v1t1, v1n, 8, 10)):
                    ntok = S * S * S
                    for (half, vt, np_) in (("a", vt0, 128), ("b", vt1, 64)):
                        dgt = dgs[f"dg{br}{half}"]
                        bia = lcb[br][0 if half == "a" else 1]
                        vp = big.tile([np_, P * P * P], BF16,
                                      tag="t_vp",
                                      name=f"vp{br}{half}")
                        nc.vector.memset(vp, 0.0)
                        vpv = vp.rearrange("p (h w d) -> p h w d", h=P, w=P, d=P)
                        nc.vector.tensor_copy(
                            out=vpv[:, 1:S + 1, 1:S + 1, 1:S + 1],
                            in_=vt.rearrange("p (h w d) -> p h w d", h=S, w=S, d=S))
                        acc = big.tile([np_, ntok], BF16,
                                       tag="t_acc",
                                       name=f"ac{br}{half}")
                        hrows = 512 // (S * S)
                        for ch in range(ntok // 512):
                            pd_ = pmm.tile([128, 512], F32, tag="mm")
                            for j, (dz, dy, dx) in enumerate(offs):
                                rhs = bass.AP(
                                    tensor=vp.tensor,
                                    offset=vp.offset + ch * hrows * P * P
                                    + dz * P * P + dy * P + dx,
                                    ap=[list(vp.ap[0]), [P * P, hrows],
                                        [P, S], [1, S]])
                                nc.tensor.matmul(pd_[0:np_, :], lhsT=dgt[:, j, :],
                                                 rhs=rhs, start=(j == 0),
                                                 stop=(j == 26))
                            nc.vector.scalar_tensor_tensor(
                                out=acc[:, ch * 512:(ch + 1) * 512],
                                in0=pd_[0:np_, :], scalar=bia,
                                in1=vt[:, ch * 512:(ch + 1) * 512],
                                op0=AL.add, op1=AL.add)
                        for mt in range(ntok // 128):
                            tp = ptr.tile([128, 128], BF16, tag="tp")
                            nc.tensor.transpose(tp[:, 0:np_],
                                                acc[:, mt * 128:(mt + 1) * 128],
                                                ident[0:np_, 0:np_])
                            vst = stg.tile([128, 128], BF16, tag="vst")
                            nc.vector.tensor_copy(out=vst[:, 0:np_], in_=tp[:, 0:np_])
                            c0 = 0 if half == "a" else 128
                            for (soff, ln, dcol) in _v49_runs(c0, np_):
                                dma(out=vn[:, mt, dcol:dcol + ln],
                                    in_=vst[:, soff:soff + ln])

        # ================= attention =================
        with tc.tile_pool(name="pS", bufs=1, space="PSUM") as pS, \
             tc.tile_pool(name="pO", bufs=1, space="PSUM") as pO, \
             tc.tile_pool(name="pex", bufs=2) as pex, \
             tc.tile_pool(name="prec", bufs=2) as prec:
            for nb in range(NQ // 512):
                for (br, kp, qp, vn, nmt) in ((2, k2p, q2p, v2n, N // 128),
                                              (1, k1p, q1p, v1n, N1 // 128)):
                    oa = pO.tile([128, 512], F32, tag="oa")
                    ob = pO.tile([128, 512], F32, tag="ob")
                    for mt in range(nmt):
                        S = pS.tile([128, 2048], F32, tag="S")
                        for h in range(4):
                            tt, r = h // 2, h % 2
                            nc.tensor.matmul(
                                S[:, h * 512:(h + 1) * 512],
                                lhsT=kp[64 * r:64 * r + 64, tt, mt * 128:(mt + 1) * 128],
                                rhs=qp[64 * r:64 * r + 64, tt, nb * 512:(nb + 1) * 512],
                                start=True, stop=True, tile_position=(64 * r, 0))
                        ex = pex.tile([128, 2048], BF16, tag="ex")
                        nc.scalar.activation(out=ex, in_=S, func=AF.Exp)
                        for h in range(4):
                            ot = oa if h < 2 else ob
                            cp = 64 * (h % 2)
                            nc.tensor.matmul(ot[cp:cp + 49, :],
                                             lhsT=vn[:, mt, h * 49:(h + 1) * 49],
                                             rhs=ex[:, h * 512:(h + 1) * 512],
                                             start=(mt == 0), stop=(mt == nmt - 1),
                                             tile_position=(0, cp))
                    # denominators -> recip -> broadcast (DRAM round-trips)
                    dta = stg.tile([128, 512], F32, tag="dta")
                    nc.vector.tensor_copy(out=dta[0:32, :], in_=oa[32:64, :])
                    nc.vector.tensor_copy(out=dta[32:64, :], in_=oa[96:128, :])
                    nc.vector.tensor_copy(out=dta[64:96, :], in_=ob[32:64, :])
                    nc.vector.tensor_copy(out=dta[96:128, :], in_=ob[96:128, :])
                    for h, r0 in enumerate((16, 48, 80, 112)):
                        dma(out=dden[br - 1, nb, h], in_=dta[r0:r0 + 1, :])
                    r16 = prec.tile([128, 16], F32, tag="r16")
                    dma(out=r16, in_=dden[br - 1, nb].rearrange("h (p k) -> (h p) k", k=16))
                    rr = prec.tile([128, 16], F32, tag="rr")
                    nc.vector.reciprocal(out=rr, in_=r16)
                    dma(out=drec[br - 1, nb].rearrange("h (p k) -> (h p) k", k=16), in_=rr)
                    reca = prec.tile([128, 512], F32, tag="reca")
                    recb = prec.tile([128, 512], F32, tag="recb")
                    for h, rt in ((0, reca), (1, reca), (2, recb), (3, recb)):
                        dma(out=rt[64 * (h % 2):64 * (h % 2) + 48, :],
                            in_=_bcast(drec[br - 1, nb, h], 48))
                    # normalize into staging, then split-DMA into ocat
                    cbase = 0 if br == 1 else C2
                    for pi, (srcp, rt) in enumerate(((oa, reca), (ob, recb))):
                        ost = stg.tile([128, 512], BF16, tag="ost")
                        for r0 in (0, 64):
                            nc.vector.scalar_tensor_tensor(
                                out=ost[r0:r0 + 48, :], in0=srcp[r0:r0 + 48, :],
                                scalar=1.0, in1=rt[r0:r0 + 48, :],
                                op0=AL.mult, op1=AL.mult)
                        for hh in range(2):
                            h = pi * 2 + hh
                            for (ct, drow, off, ln) in _cat_runs(cbase + h * 48, 48):
                                dma(out=ocat[drow:drow + ln, ct,
                                             nb * 512:(nb + 1) * 512],
                                    in_=ost[64 * hh + off:64 * hh + off + ln, :])

        # ================= proj + LN2 + MLP + output =================
        with tc.tile_pool(name="pG", bufs=1) as pG, \
             tc.tile_pool(name="pmm2", bufs=3, space="PSUM") as pmm2, \
             tc.tile_pool(name="ptr2", bufs=2, space="PSUM") as ptr2:
            pw_sb = mat_sb("pw", CT, C, pG)
            f1w_sb = mat_sb("f1w", CT, HID, pG)
            f2w_sb = mat_sb("f2w", 12, C, pG)
            zt = pG.tile([128, CT, NQ], F32, tag="zt")
            xct_sb = pG.tile([128, CT, NQ], F32, tag="xctG")
            dma(out=xct_sb, in_=d["xct"].rearrange("(t p) n -> p t n", p=128))
            for mt in range(CT):
                for ch in range(NQ // 512):
                    ps = pmm2.tile([128, 512], F32, tag="mm")
                    for ct in range(CT):
                        nc.tensor.matmul(ps, lhsT=pw_sb[:, ct, mt * 128:(mt + 1) * 128],
                                         rhs=ocat[:, ct, ch * 512:(ch + 1) * 512],
                                         start=(ct == 0), stop=(ct == CT - 1))
                    nc.vector.scalar_tensor_tensor(
                        out=zt[:, mt, ch * 512:(ch + 1) * 512], in0=ps,
                        scalar=pb_sb[:, mt:mt + 1],
                        in1=xct_sb[:, mt, ch * 512:(ch + 1) * 512],
                        op0=AL.add, op1=AL.add)
            a3, b3 = t_ln_rows(lambda ct: zt[:, ct, :], NQ, "l2", pG,
                               src_f32=True, out_dt=F32)
            xm = pG.tile([128, CT, NQ], BF16, tag="xm")
            tmp3 = pG.tile([128, NQ], F32, tag="tmp3")
            for ct in range(CT):
                nc.vector.tensor_mul(tmp3, zt[:, ct, :], a3)
                nc.vector.tensor_add(tmp3, tmp3, b3)
                nc.scalar.activation(out=xm[:, ct, :], in_=tmp3, func=AF.Identity,
                                     bias=ln2b_sb[:, ct:ct + 1],
                                     scale=ln2w_sb[:, ct:ct + 1])
            h1 = pG.tile([128, 12, NQ], BF16, tag="h1")
            for mt in range(12):
                for ch in range(NQ // 512):
                    ps = pmm2.tile([128, 512], F32, tag="mm")
                    for ct in range(CT):
                        nc.tensor.matmul(ps, lhsT=f1w_sb[:, ct, mt * 128:(mt + 1) * 128],
                                         rhs=xm[:, ct, ch * 512:(ch + 1) * 512],
                                         start=(ct == 0), stop=(ct == CT - 1))
                    nc.scalar.activation(out=h1[:, mt, ch * 512:(ch + 1) * 512],
                                         in_=ps, func=AF.Gelu,
                                         bias=f1b_sb[:, mt:mt + 1])
            for mt in range(CT):
                for ch in range(NQ // 512):
                    ps = pmm2.tile([128, 512], F32, tag="mm")
                    for kt in range(12):
                        nc.tensor.matmul(ps, lhsT=f2w_sb[:, kt, mt * 128:(mt + 1) * 128],
                                         rhs=h1[:, kt, ch * 512:(ch + 1) * 512],
                                         start=(kt == 0), stop=(kt == 11))
                    nc.vector.scalar_tensor_tensor(
                        out=zt[:, mt, ch * 512:(ch + 1) * 512], in0=ps,
                        scalar=f2b_sb[:, mt:mt + 1],
                        in1=zt[:, mt, ch * 512:(ch + 1) * 512],
                        op0=AL.add, op1=AL.add)
            for nt in range(NQ // 128):
                for ct in range(CT):
                    tp = ptr2.tile([128, 128], F32, tag="tp2")
                    nc.tensor.transpose(tp, zt[:, ct, nt * 128:(nt + 1) * 128], identf)
                    ots = stg.tile([128, 128], F32, tag="ots")
                    nc.vector.tensor_copy(out=ots, in_=tp)
                    dma(out=out_d[nt * 128:(nt + 1) * 128, ct * 128:(ct + 1) * 128],
                        in_=ots)


_PROG = None


def _get_program():
    global _PROG
    if _PROG is None:
        _PROG = build_program()
    return _PROG


def _diag(w):
    """[n, 27] weights -> [n, 27, n] per-offset diagonal matrices (bf16)."""
    n = w.shape[0]
    out = np.zeros((n, 27, n), BF)
    idx = np.arange(n)
    for j in range(27):
        out[idx, j, idx] = w[:, j].astype(BF)
    return out


def kernel(x, ln1_w, ln1_b, q_w, sr1_w, sr1_b, n1_w, n1_b, sr2_w, sr2_b,
           n2_w, n2_b, kv1_w, kv2_w, lc1_w, lc1_b, lc2_w, lc2_b,
           proj_w, proj_b, ln2_w, ln2_b, fc1_w, fc1_b, fc2_w, fc2_b,
           H, W, D):
    f = lambda a: np.asarray(a, np.float32)
    x = f(x)
    ln1_w, ln1_b = f(ln1_w), f(ln1_b)
    qs = HD ** -0.5
    lc1 = f(lc1_w).reshape(C2, 27)
    lc2 = f(lc2_w).reshape(C2, 27)

    wm = {
        "qw": np.ascontiguousarray((f(q_w) * ln1_w[None, :]).T * qs).astype(BF),
        "qb": (f(q_w) @ ln1_b * qs).astype(np.float32),
        "s2w": np.ascontiguousarray((f(sr2_w)[:, :, 0, 0, 0] * ln1_w[None, :]).T).astype(BF),
        "s2b": (f(sr2_b) + f(sr2_w)[:, :, 0, 0, 0] @ ln1_b).astype(np.float32),
        "s1w": np.ascontiguousarray(
            (f(sr1_w) * ln1_w[None, :, None, None, None])
            .transpose(2, 3, 4, 1, 0).reshape(8, C, C)).astype(BF),
        "s1b": (f(sr1_b) + np.einsum("ocijk,c->o", f(sr1_w), ln1_b)).astype(np.float32),
        "n1w": f(n1_w), "n1b": f(n1_b), "n2w": f(n2_w), "n2b": f(n2_b),
        "kv1w": np.ascontiguousarray(f(kv1_w).T).astype(BF),
        "kv2w": np.ascontiguousarray(f(kv2_w).T).astype(BF),
        "dg1a": _diag(lc1[0:128]), "dg1b": _diag(lc1[128:192]),
        "dg2a": _diag(lc2[0:128]), "dg2b": _diag(lc2[128:192]),
        "lc1b": f(lc1_b), "lc2b": f(lc2_b),
        "pw": np.ascontiguousarray(f(proj_w).T).astype(BF),
        "pb": f(proj_b),
        "ln2w": f(ln2_w), "ln2b": f(ln2_b),
        "f1w": np.ascontiguousarray(f(fc1_w).T).astype(BF),
        "f1b": f(fc1_b),
        "f2w": np.ascontiguousarray(f(fc2_w).T).astype(BF),
        "f2b": f(fc2_b),
    }

    in_maps = []
    for core in range(8):
        b, qc = core // 4, core % 4
        xtb = x[b].T
        m = dict(wm)
        m["xt"] = np.ascontiguousarray(xtb).astype(BF)
        m["xct"] = np.ascontiguousarray(xtb[:, qc * NQ:(qc + 1) * NQ]).astype(np.float32)
        in_maps.append(m)

    nc = _get_program()
    res = run_bass_kernel_spmd(nc, in_maps, list(range(8)))

    out = np.empty((B, N, C), np.float32)
    for core in range(8):
        b, qc = core // 4, core % 4
        out[b, qc * NQ:(qc + 1) * NQ, :] = res.results[core]["out"]
    return out
